# revision 7
# baseline (speedup 1.0000x reference)
"""Trainium2 Bass kernel for EnergyIrrepModulation.

Computes out[m, e, d] = x[m, d] * gates_full[e, d] where
gates = MLP(e_feat) : [nE, n_copies], expanded to [nE, D] via the static
irrep index map for IRREPS = [(64, 1), (32, 3), (16, 5)].

Sharding: data-parallel over M (4096 rows -> 512 rows per core, 8 cores).
Gates/MLP params are replicated; each core redundantly computes the tiny MLP.

Per-core device plan:
  1. MLP on the tensor engine (PE), biases+ReLU on the scalar engine (ACT).
     All operands are pre-transposed on the host so no on-device transposes
     are needed (e_feat is passed as e_featT [64, 100]).
  2. Expand gates [100, 112] -> gates_full [100, 240] with broadcast-read
     copies (free-dim gather per the irrep block structure).
  3. Broadcast gates_full across all 128 partitions with ones-vector
     matmuls on PE (PSUM -> SBUF copies on ACT). Zero extra HBM traffic.
  4. Main loop: stream x tiles [128, 240], multiply against the broadcast
     gates on the vector engine (stride-0 read of x over the e axis),
     DMA the [128, 6000] output chunks to HBM. This is HBM-write-bound:
     ~49 MB per core at ~360 GB/s.
"""

import sys
from contextlib import ExitStack

import numpy as np

try:
    import concourse.bass as bass  # noqa: F401
except ImportError:  # pragma: no cover
    sys.path.insert(0, "/opt/trn_rl_repo")
    import concourse.bass as bass

import concourse.bacc as bacc
import concourse.tile as tile
from concourse import mybir
from concourse.bass_utils import run_bass_kernel_spmd

FP32 = mybir.dt.float32

M, D = 4096, 240
NE, E_DIM, HIDDEN, NCOP = 100, 64, 256, 112
N_CORES = 8
MC = M // N_CORES          # 512 rows per core
MT = MC // 128             # 4 m-tiles of 128 rows
EC = 25                    # e-chunk size
NEC = NE // EC             # 4 e-chunks
CHUNK = EC * D             # 6000 elements per out chunk
BCN = 500                  # bcast matmul free-dim size (<= 512 psum bank)
NBC = CHUNK // BCN         # 12 bcast matmuls per e-chunk

_CACHE = {}


def _build_program():
    nc = bacc.Bacc(None, target_bir_lowering=False, debug=False)

    x_d = nc.dram_tensor("x", [MC, D], FP32, kind="ExternalInput")
    eT_d = nc.dram_tensor("eT", [E_DIM, NE], FP32, kind="ExternalInput")
    w1_d = nc.dram_tensor("W1", [E_DIM, HIDDEN], FP32, kind="ExternalInput")
    w2_d = nc.dram_tensor("W2", [HIDDEN, HIDDEN], FP32, kind="ExternalInput")
    w3_d = nc.dram_tensor("W3", [HIDDEN, NCOP], FP32, kind="ExternalInput")
    b1_d = nc.dram_tensor("b1", [HIDDEN, 1], FP32, kind="ExternalInput")
    b2_d = nc.dram_tensor("b2", [HIDDEN, 1], FP32, kind="ExternalInput")
    b3_d = nc.dram_tensor("b3", [1, NCOP], FP32, kind="ExternalInput")
    out_d = nc.dram_tensor("out", [MC, NE * D], FP32, kind="ExternalOutput")

    with tile.TileContext(nc) as tc, ExitStack() as ctx:
        const_pool = ctx.enter_context(tc.tile_pool(name="const", bufs=1))
        mlp_pool = ctx.enter_context(tc.tile_pool(name="mlp", bufs=1))
        psum_mlp = ctx.enter_context(
            tc.tile_pool(name="psum_mlp", bufs=2, space="PSUM")
        )
        psum_bc = ctx.enter_context(
            tc.tile_pool(name="psum_bc", bufs=4, space="PSUM")
        )
        gbc_pool = ctx.enter_context(tc.tile_pool(name="gbc", bufs=1))
        x_pool = ctx.enter_context(tc.tile_pool(name="xin", bufs=2))
        out_pool = ctx.enter_context(tc.tile_pool(name="out", bufs=2))

        # ---- load tiny params ----
        eT_t = const_pool.tile([E_DIM, NE], FP32)
        nc.gpsimd.dma_start(out=eT_t[:], in_=eT_d[:, :])
        w1_t = const_pool.tile([E_DIM, HIDDEN], FP32)
        nc.gpsimd.dma_start(out=w1_t[:], in_=w1_d[:, :])
        w2a_t = const_pool.tile([128, HIDDEN], FP32)
        nc.gpsimd.dma_start(out=w2a_t[:], in_=w2_d[0:128, :])
        w2b_t = const_pool.tile([128, HIDDEN], FP32)
        nc.gpsimd.dma_start(out=w2b_t[:], in_=w2_d[128:256, :])
        w3a_t = const_pool.tile([128, NCOP], FP32)
        nc.gpsimd.dma_start(out=w3a_t[:], in_=w3_d[0:128, :])
        w3b_t = const_pool.tile([128, NCOP], FP32)
        nc.gpsimd.dma_start(out=w3b_t[:], in_=w3_d[128:256, :])
        b1_t = const_pool.tile([128, 2], FP32)
        nc.gpsimd.dma_start(out=b1_t[:, 0:1], in_=b1_d[0:128, :])
        nc.gpsimd.dma_start(out=b1_t[:, 1:2], in_=b1_d[128:256, :])
        b2_t = const_pool.tile([128, 2], FP32)
        nc.gpsimd.dma_start(out=b2_t[:, 0:1], in_=b2_d[0:128, :])
        nc.gpsimd.dma_start(out=b2_t[:, 1:2], in_=b2_d[128:256, :])
        b3_t = const_pool.tile([1, NCOP], FP32)
        nc.gpsimd.dma_start(out=b3_t[:], in_=b3_d[:, :])

        ones_t = const_pool.tile([1, 128], FP32)
        nc.vector.memset(ones_t[:], 1.0)

        # ---- MLP: h1T = relu(W1^T e_featT + b1) as two [128, 100] tiles ----
        relu = mybir.ActivationFunctionType.Relu
        h1T = []
        for c in range(2):
            ps = psum_mlp.tile([128, NE], FP32)
            nc.tensor.matmul(
                ps[:], w1_t[:, c * 128 : (c + 1) * 128], eT_t[:],
                start=True, stop=True,
            )
            h = mlp_pool.tile([128, NE], FP32, tag=f"h1T{c}")
            nc.scalar.activation(
                h[:], ps[:], relu, bias=b1_t[:, c : c + 1]
            )
            h1T.append(h)

        # ---- h2T = relu(W2^T h1T + b2) ----
        h2T = []
        for c in range(2):
            ps = psum_mlp.tile([128, NE], FP32)
            nc.tensor.matmul(
                ps[:], w2a_t[:, c * 128 : (c + 1) * 128], h1T[0][:],
                start=True, stop=False,
            )
            nc.tensor.matmul(
                ps[:], w2b_t[:, c * 128 : (c + 1) * 128], h1T[1][:],
                start=False, stop=True,
            )
            h = mlp_pool.tile([128, NE], FP32, tag=f"h2T{c}")
            nc.scalar.activation(
                h[:], ps[:], relu, bias=b2_t[:, c : c + 1]
            )
            h2T.append(h)

        # ---- gates = h2 @ W3 + b3 : psum [100, 112], partition = e ----
        psg = psum_mlp.tile([NE, NCOP], FP32)
        nc.tensor.matmul(psg[:], h2T[0][:], w3a_t[:], start=True, stop=False)
        nc.tensor.matmul(psg[:], h2T[1][:], w3b_t[:], start=False, stop=False)
        # += ones[100,1] @ b3[1,112] to add the bias along the free dim
        nc.tensor.matmul(
            psg[:], ones_t[0:1, 0:NE], b3_t[:], start=False, stop=True
        )
        gates_t = mlp_pool.tile([NE, NCOP], FP32)
        nc.scalar.copy(gates_t[:], psg[:])

        # ---- expand gates [100, 112] -> gates_full [100, 240] ----
        gfull_t = mlp_pool.tile([NE, D], FP32)
        nc.vector.tensor_copy(gfull_t[:, 0:64], gates_t[:, 0:64])
        nc.vector.tensor_copy(
            gfull_t[:, 64:160].rearrange("p (i k) -> p i k", k=3),
            gates_t[:, 64:96].unsqueeze(2).to_broadcast((NE, 32, 3)),
        )
        nc.vector.tensor_copy(
            gfull_t[:, 160:240].rearrange("p (i k) -> p i k", k=5),
            gates_t[:, 96:112].unsqueeze(2).to_broadcast((NE, 16, 5)),
        )

        # ---- broadcast to all 128 partitions via ones-matmuls on PE ----
        # Flatten each e-chunk of gates_full onto partition 0 (SBUF->SBUF
        # DMA), then PE-matmul ones[1,128]^T @ flat[1, 500] to replicate.
        stage_pool = ctx.enter_context(tc.tile_pool(name="stage", bufs=1))
        gbc_t = gbc_pool.tile([128, NE * D], FP32)
        for ec in range(NEC):
            st = stage_pool.tile([1, CHUNK], FP32)
            nc.gpsimd.dma_start(
                out=st[:], in_=gfull_t[ec * EC : (ec + 1) * EC, :]
            )
            for n in range(NBC):
                ps = psum_bc.tile([128, BCN], FP32)
                nc.tensor.matmul(
                    ps[:],
                    ones_t[0:1, :],
                    st[0:1, n * BCN : (n + 1) * BCN],
                    start=True, stop=True,
                )
                off = ec * CHUNK + n * BCN
                nc.scalar.copy(gbc_t[:, off : off + BCN], ps[:])

        # ---- main loop: out[m, e, d] = x[m, d] * gbc[(e, d)] ----
        gbc_v = gbc_t[:].rearrange("p (e d) -> p e d", d=D)
        for mt in range(MT):
            x_t = x_pool.tile([128, D], FP32)
            nc.gpsimd.dma_start(out=x_t[:], in_=x_d[mt * 128 : (mt + 1) * 128, :])
            x_v = x_t[:].unsqueeze(1).to_broadcast((128, EC, D))
            for ec in range(NEC):
                o_t = out_pool.tile([128, CHUNK], FP32)
                nc.vector.tensor_mul(
                    o_t[:].rearrange("p (e d) -> p e d", d=D),
                    x_v,
                    gbc_v[:, ec * EC : (ec + 1) * EC, :],
                )
                nc.sync.dma_start(
                    out=out_d[mt * 128 : (mt + 1) * 128, ec * CHUNK : (ec + 1) * CHUNK],
                    in_=o_t[:],
                )

    nc.compile()
    return nc


def _marshal(inputs):
    f32 = lambda a: np.ascontiguousarray(np.asarray(a, dtype=np.float32))
    x = f32(inputs["x"])
    shared = {
        "eT": f32(np.asarray(inputs["e_feat"]).T),
        "W1": f32(inputs["W1"]),
        "W2": f32(inputs["W2"]),
        "W3": f32(inputs["W3"]),
        "b1": f32(inputs["b1"]).reshape(HIDDEN, 1),
        "b2": f32(inputs["b2"]).reshape(HIDDEN, 1),
        "b3": f32(inputs["b3"]).reshape(1, NCOP),
    }
    return [
        {"x": x[i * MC : (i + 1) * MC], **shared} for i in range(N_CORES)
    ]


def get_program():
    if "nc" not in _CACHE:
        _CACHE["nc"] = _build_program()
    return _CACHE["nc"]


def run(inputs, trace=False, **kwargs):
    """Run on 8 cores; returns (out [M, NE, D], BassKernelResults)."""
    nc = get_program()
    in_maps = _marshal(inputs)
    res = run_bass_kernel_spmd(
        nc, in_maps, core_ids=list(range(N_CORES)), trace=trace, **kwargs
    )
    out = np.concatenate(
        [np.asarray(res.results[i]["out"]).reshape(MC, NE, D) for i in range(N_CORES)],
        axis=0,
    )
    return out, res


def kernel(**inputs) -> np.ndarray:
    out, _ = run(inputs)
    return out


# revision 9
# speedup vs baseline: 1.2281x; 1.2281x over previous
"""Trainium2 Bass kernel for EnergyIrrepModulation.

Computes out[m, e, d] = x[m, d] * gates_full[e, d] where
gates = MLP(e_feat) : [nE, n_copies], expanded to [nE, D] via the static
irrep index map for IRREPS = [(64, 1), (32, 3), (16, 5)].

Sharding: data-parallel over M (4096 rows -> 512 rows per core, 8 cores).
Gates/MLP params are replicated; each core redundantly computes the tiny MLP.

Per-core device plan:
  1. MLP on the tensor engine (PE), biases+ReLU on the scalar engine (ACT).
     All operands are pre-transposed on the host so no on-device transposes
     are needed (e_feat is passed as e_featT [64, 100]).
  2. Expand gates [100, 112] -> gates_full [100, 240] with broadcast-read
     copies (free-dim gather per the irrep block structure).
  3. Broadcast gates_full across all 128 partitions with ones-vector
     matmuls on PE (PSUM -> SBUF copies on ACT). Zero extra HBM traffic.
  4. Main loop: stream x tiles [128, 240], multiply against the broadcast
     gates on the vector engine (stride-0 read of x over the e axis),
     DMA the [128, 6000] output chunks to HBM. This is HBM-write-bound:
     ~49 MB per core at ~360 GB/s.
"""

import sys
from contextlib import ExitStack

import numpy as np

try:
    import concourse.bass as bass  # noqa: F401
except ImportError:  # pragma: no cover
    sys.path.insert(0, "/opt/trn_rl_repo")
    import concourse.bass as bass

import concourse.bacc as bacc
import concourse.tile as tile
from concourse import mybir
from concourse.bass_utils import run_bass_kernel_spmd

FP32 = mybir.dt.float32

M, D = 4096, 240
NE, E_DIM, HIDDEN, NCOP = 100, 64, 256, 112
N_CORES = 8
MC = M // N_CORES          # 512 rows per core
MT = MC // 128             # 4 m-tiles of 128 rows
EC = 25                    # e-chunk size
NEC = NE // EC             # 4 e-chunks
CHUNK = EC * D             # 6000 elements per out chunk
BCN = 500                  # bcast matmul free-dim size (<= 512 psum bank)
NBC = CHUNK // BCN         # 12 bcast matmuls per e-chunk

_CACHE = {}


def _build_program():
    nc = bacc.Bacc(None, target_bir_lowering=False, debug=False)

    x_d = nc.dram_tensor("x", [MC, D], FP32, kind="ExternalInput")
    eT_d = nc.dram_tensor("eT", [E_DIM, NE], FP32, kind="ExternalInput")
    w1_d = nc.dram_tensor("W1", [E_DIM, HIDDEN], FP32, kind="ExternalInput")
    w2_d = nc.dram_tensor("W2", [HIDDEN, HIDDEN], FP32, kind="ExternalInput")
    w3_d = nc.dram_tensor("W3", [HIDDEN, NCOP], FP32, kind="ExternalInput")
    b1_d = nc.dram_tensor("b1", [HIDDEN, 1], FP32, kind="ExternalInput")
    b2_d = nc.dram_tensor("b2", [HIDDEN, 1], FP32, kind="ExternalInput")
    b3_d = nc.dram_tensor("b3", [1, NCOP], FP32, kind="ExternalInput")
    out_d = nc.dram_tensor("out", [MC, NE * D], FP32, kind="ExternalOutput")

    with tile.TileContext(nc) as tc, ExitStack() as ctx:
        const_pool = ctx.enter_context(tc.tile_pool(name="const", bufs=1))
        mlp_pool = ctx.enter_context(tc.tile_pool(name="mlp", bufs=1))
        psum_mlp = ctx.enter_context(
            tc.tile_pool(name="psum_mlp", bufs=2, space="PSUM")
        )
        psum_bc = ctx.enter_context(
            tc.tile_pool(name="psum_bc", bufs=4, space="PSUM")
        )
        gbc_pool = ctx.enter_context(tc.tile_pool(name="gbc", bufs=1))
        x_pool = ctx.enter_context(tc.tile_pool(name="xin", bufs=2))
        out_pool = ctx.enter_context(tc.tile_pool(name="out", bufs=2))

        # ---- load tiny params ----
        eT_t = const_pool.tile([E_DIM, NE], FP32)
        nc.gpsimd.dma_start(out=eT_t[:], in_=eT_d[:, :])
        w1_t = const_pool.tile([E_DIM, HIDDEN], FP32)
        nc.gpsimd.dma_start(out=w1_t[:], in_=w1_d[:, :])
        w2a_t = const_pool.tile([128, HIDDEN], FP32)
        nc.gpsimd.dma_start(out=w2a_t[:], in_=w2_d[0:128, :])
        w2b_t = const_pool.tile([128, HIDDEN], FP32)
        nc.gpsimd.dma_start(out=w2b_t[:], in_=w2_d[128:256, :])
        w3a_t = const_pool.tile([128, NCOP], FP32)
        nc.gpsimd.dma_start(out=w3a_t[:], in_=w3_d[0:128, :])
        w3b_t = const_pool.tile([128, NCOP], FP32)
        nc.gpsimd.dma_start(out=w3b_t[:], in_=w3_d[128:256, :])
        b1_t = const_pool.tile([128, 2], FP32)
        nc.gpsimd.dma_start(out=b1_t[:, 0:1], in_=b1_d[0:128, :])
        nc.gpsimd.dma_start(out=b1_t[:, 1:2], in_=b1_d[128:256, :])
        b2_t = const_pool.tile([128, 2], FP32)
        nc.gpsimd.dma_start(out=b2_t[:, 0:1], in_=b2_d[0:128, :])
        nc.gpsimd.dma_start(out=b2_t[:, 1:2], in_=b2_d[128:256, :])
        b3_t = const_pool.tile([1, NCOP], FP32)
        nc.gpsimd.dma_start(out=b3_t[:], in_=b3_d[:, :])

        ones_t = const_pool.tile([1, 128], FP32)
        nc.vector.memset(ones_t[:], 1.0)

        # ---- MLP: h1T = relu(W1^T e_featT + b1) as two [128, 100] tiles ----
        relu = mybir.ActivationFunctionType.Relu
        h1T = []
        for c in range(2):
            ps = psum_mlp.tile([128, NE], FP32)
            nc.tensor.matmul(
                ps[:], w1_t[:, c * 128 : (c + 1) * 128], eT_t[:],
                start=True, stop=True,
            )
            h = mlp_pool.tile([128, NE], FP32, tag=f"h1T{c}")
            nc.scalar.activation(
                h[:], ps[:], relu, bias=b1_t[:, c : c + 1]
            )
            h1T.append(h)

        # ---- h2T = relu(W2^T h1T + b2) ----
        h2T = []
        for c in range(2):
            ps = psum_mlp.tile([128, NE], FP32)
            nc.tensor.matmul(
                ps[:], w2a_t[:, c * 128 : (c + 1) * 128], h1T[0][:],
                start=True, stop=False,
            )
            nc.tensor.matmul(
                ps[:], w2b_t[:, c * 128 : (c + 1) * 128], h1T[1][:],
                start=False, stop=True,
            )
            h = mlp_pool.tile([128, NE], FP32, tag=f"h2T{c}")
            nc.scalar.activation(
                h[:], ps[:], relu, bias=b2_t[:, c : c + 1]
            )
            h2T.append(h)

        # ---- gates = h2 @ W3 + b3 : psum [100, 112], partition = e ----
        psg = psum_mlp.tile([NE, NCOP], FP32)
        nc.tensor.matmul(psg[:], h2T[0][:], w3a_t[:], start=True, stop=False)
        nc.tensor.matmul(psg[:], h2T[1][:], w3b_t[:], start=False, stop=False)
        # += ones[100,1] @ b3[1,112] to add the bias along the free dim
        nc.tensor.matmul(
            psg[:], ones_t[0:1, 0:NE], b3_t[:], start=False, stop=True
        )
        gates_t = mlp_pool.tile([NE, NCOP], FP32)
        nc.scalar.copy(gates_t[:], psg[:])

        # ---- expand gates [100, 112] -> gates_full [100, 240] ----
        gfull_t = mlp_pool.tile([NE, D], FP32)
        nc.vector.tensor_copy(gfull_t[:, 0:64], gates_t[:, 0:64])
        nc.vector.tensor_copy(
            gfull_t[:, 64:160].rearrange("p (i k) -> p i k", k=3),
            gates_t[:, 64:96].unsqueeze(2).to_broadcast((NE, 32, 3)),
        )
        nc.vector.tensor_copy(
            gfull_t[:, 160:240].rearrange("p (i k) -> p i k", k=5),
            gates_t[:, 96:112].unsqueeze(2).to_broadcast((NE, 16, 5)),
        )

        # ---- broadcast to all 128 partitions via GPSIMD ----
        # Flatten each e-chunk of gates_full onto partition 0 (SBUF->SBUF
        # DMA), then partition_broadcast replicates it to 128 partitions.
        stage_pool = ctx.enter_context(tc.tile_pool(name="stage", bufs=2))
        gbc_t = gbc_pool.tile([128, NE * D], FP32)
        for ec in range(NEC):
            st = stage_pool.tile([1, CHUNK], FP32)
            nc.gpsimd.dma_start(
                out=st[:], in_=gfull_t[ec * EC : (ec + 1) * EC, :]
            )
            nc.gpsimd.partition_broadcast(
                gbc_t[:, ec * CHUNK : (ec + 1) * CHUNK], st[0:1, :]
            )

        # ---- main loop: out[m, e, d] = x[m, d] * gbc[(e, d)] ----
        gbc_v = gbc_t[:].rearrange("p (e d) -> p e d", d=D)
        for mt in range(MT):
            x_t = x_pool.tile([128, D], FP32)
            nc.gpsimd.dma_start(out=x_t[:], in_=x_d[mt * 128 : (mt + 1) * 128, :])
            x_v = x_t[:].unsqueeze(1).to_broadcast((128, EC, D))
            for ec in range(NEC):
                o_t = out_pool.tile([128, CHUNK], FP32)
                nc.vector.tensor_mul(
                    o_t[:].rearrange("p (e d) -> p e d", d=D),
                    x_v,
                    gbc_v[:, ec * EC : (ec + 1) * EC, :],
                )
                # split the store across both HWDGE rings (SP + ACT)
                half = CHUNK // 2
                base = ec * CHUNK
                nc.sync.dma_start(
                    out=out_d[mt * 128 : (mt + 1) * 128, base : base + half],
                    in_=o_t[:, 0:half],
                )
                nc.scalar.dma_start(
                    out=out_d[mt * 128 : (mt + 1) * 128, base + half : base + CHUNK],
                    in_=o_t[:, half:CHUNK],
                )

    nc.compile()
    return nc


def _marshal(inputs):
    f32 = lambda a: np.ascontiguousarray(np.asarray(a, dtype=np.float32))
    x = f32(inputs["x"])
    shared = {
        "eT": f32(np.asarray(inputs["e_feat"]).T),
        "W1": f32(inputs["W1"]),
        "W2": f32(inputs["W2"]),
        "W3": f32(inputs["W3"]),
        "b1": f32(inputs["b1"]).reshape(HIDDEN, 1),
        "b2": f32(inputs["b2"]).reshape(HIDDEN, 1),
        "b3": f32(inputs["b3"]).reshape(1, NCOP),
    }
    return [
        {"x": x[i * MC : (i + 1) * MC], **shared} for i in range(N_CORES)
    ]


def get_program():
    if "nc" not in _CACHE:
        _CACHE["nc"] = _build_program()
    return _CACHE["nc"]


def run(inputs, trace=False, **kwargs):
    """Run on 8 cores; returns (out [M, NE, D], BassKernelResults)."""
    nc = get_program()
    in_maps = _marshal(inputs)
    res = run_bass_kernel_spmd(
        nc, in_maps, core_ids=list(range(N_CORES)), trace=trace, **kwargs
    )
    out = np.concatenate(
        [np.asarray(res.results[i]["out"]).reshape(MC, NE, D) for i in range(N_CORES)],
        axis=0,
    )
    return out, res


def kernel(**inputs) -> np.ndarray:
    out, _ = run(inputs)
    return out
